# revision 40
# baseline (speedup 1.0000x reference)
"""Bahdanau attention Trainium2 kernel.

reference:
    proj_v = values @ W1 + b1            # [B,T,U]
    proj_q = (query @ W2 + b2)[:,None,:] # [B,1,U]
    score  = tanh(proj_v + proj_q) @ V + bv
    attn   = softmax(score, axis=1)      # [B,T,1]
    context= sum(attn * values, axis=1)  # [B,D]

Sharding: data-parallel over batch, B=64 -> 8 batches per core on 8 cores.
Weights replicated.

Per-core dataflow (b = 8 batches, T=2048, D=U=512):
  - values arrive twice: fp32 natural [t, d] (context operand, read as f32r)
    and host-prepared bf16 transposed [d, t] (proj moving operand).  The
    transpose is done on the host because the on-device DMA-xbar transpose
    does not honor its semaphore waits reliably (verified race on HW).
  - proj: PE matmul, W1 (host-cast bf16) stationary, valuesT bf16 moving
    -> PSUM [u, t] tiles; 1 cycle/row.
  - proj_q (query @ W2, f32r) computed once; (b1+b2) added via DVE
    tensor_scalar into a per-(u, batch) bias table; the bias is applied by
    the ACT Tanh bias operand (fused add+tanh, PSUM -> SBUF f32r).
  - score: PE matmul, V column (f32r) stationary, tanh tile moving -> [1, t].
  - softmax without max subtraction (|score| <= ||V||_1 ~ 18, exp is safe in
    fp32; bv cancels in softmax and is ignored).  ACT Exp computes E and its
    accum_out gives the partial sums of Z for free.
  - E re-laid [1, T] -> [t, ts] columns with 16 K=1 rank-1 PE matmuls
    (cheaper and race-free vs a strided-DMA roundtrip); context: PE matmul,
    E column stationary, values fp32-as-f32r moving -> [1, D]; x(1/Z) on ACT.
  - DMA spread: vT on the SP HWDGE ring, values fp32 chunks alternating
    SWDGE (gpsimd) / Act HWDGE ring, setup tinies on SWDGE first.
"""

import sys

if "/opt/trn_rl_repo" not in sys.path:
    sys.path.insert(0, "/opt/trn_rl_repo")

import numpy as np

B, T, D, U = 64, 2048, 512, 512
NCORES = 8
BPC = B // NCORES  # batches per core

P = 128
DS = D // P   # 4  d slices
US = U // P   # 4  u slices
TS = T // P   # 16 t slices of 128
TCH = 512     # t chunk for matmul free dim
TC = T // TCH  # 4
DBG_B = 1

_cache = {}


def _build_nc(debug_dump=False):
    import concourse.bass as bass
    import concourse.tile as tile
    from concourse import bacc
    from concourse import mybir
    from concourse.bass import ts

    f32 = mybir.dt.float32
    f32r = mybir.dt.float32r
    bf16 = mybir.dt.bfloat16
    Act = mybir.ActivationFunctionType

    nc = bacc.Bacc("TRN2", target_bir_lowering=False, debug=False)

    values = nc.dram_tensor("values", [BPC, T, D], f32, kind="ExternalInput")
    values_b16T = nc.dram_tensor(
        "values_b16T", [BPC, D, T], bf16, kind="ExternalInput"
    )
    qTh = nc.dram_tensor("qTh", [P, DS * BPC], f32, kind="ExternalInput")
    vcolh = nc.dram_tensor("vcolh", [P, US], f32, kind="ExternalInput")
    b12h = nc.dram_tensor("b12h", [P, US], f32, kind="ExternalInput")
    W1b16 = nc.dram_tensor("W1b16", [D, U], bf16, kind="ExternalInput")
    W2 = nc.dram_tensor("W2", [D, U], f32, kind="ExternalInput")
    ctx_out = nc.dram_tensor("context", [BPC, D], f32, kind="ExternalOutput")
    attn_out = nc.dram_tensor("attn", [BPC, T, 1], f32, kind="ExternalOutput")
    if debug_dump:
        vT_out = nc.dram_tensor("vT_dbg", [P, DS, T], bf16, kind="ExternalOutput")
        th_out = nc.dram_tensor("th_dbg", [TC, US, P, TCH], f32, kind="ExternalOutput")
        e_out = nc.dram_tensor("e_dbg", [BPC, T], f32, kind="ExternalOutput")
        bias_out = nc.dram_tensor("bias_dbg", [P, US * BPC], f32, kind="ExternalOutput")

    with tile.TileContext(nc) as tc:
        with (
            tc.tile_pool(name="consts", bufs=1) as consts,
            tc.tile_pool(name="vf32", bufs=2) as vf32p,
            tc.tile_pool(name="vT", bufs=2) as vTp,
            tc.tile_pool(name="th", bufs=8) as thp,
            tc.tile_pool(name="small", bufs=2) as smallp,
            tc.tile_pool(name="eb", bufs=10) as ebp,
            tc.tile_pool(name="ecol", bufs=2) as ecolp,
            tc.tile_pool(name="ppsum", bufs=3, space="PSUM") as ppsum,
            tc.tile_pool(name="spsum", bufs=2, space="PSUM") as spsum,
            tc.tile_pool(name="cpsum", bufs=2, space="PSUM") as cpsum,
            tc.tile_pool(name="epsum", bufs=1, space="PSUM") as epsum,
        ):
            # ---- weights / setup ----
            w1b = consts.tile([P, DS, U], bf16)
            w1src = W1b16.rearrange("(s p) u -> p s u", p=P)
            for us_ in range(US):
                nc.scalar.dma_start(
                    out=w1b[:, :, ts(us_, P)], in_=w1src[:, :, ts(us_, P)]
                )

            vcol = consts.tile([P, US], f32r)
            qT = consts.tile([P, DS, BPC], f32r)
            b12c = consts.tile([P, US], f32)
            nc.gpsimd.dma_start(out=vcol, in_=vcolh[:, :].bitcast(f32r))
            nc.gpsimd.dma_start(
                out=qT, in_=qTh.rearrange("p (s b) -> p s b", b=BPC).bitcast(f32r)
            )
            nc.gpsimd.dma_start(out=b12c, in_=b12h[:, :])
            w2f = consts.tile([P, DS, U], f32r)
            nc.gpsimd.dma_start(
                out=w2f, in_=W2.rearrange("(s p) u -> p s u", p=P).bitcast(f32r)
            )
            ones8f = consts.tile([1, BPC], f32)
            nc.vector.memset(ones8f, 1.0)

            # bias_all[u, (us,b)] = proj_q[u, b] + (b1+b2)[u]
            bias_all = consts.tile([P, US * BPC], f32)
            qp = cpsum.tile([P, US * BPC], f32, tag="cps")
            for us_ in range(US):
                for ds_ in range(DS):
                    nc.tensor.matmul(
                        qp[:, us_ * BPC : (us_ + 1) * BPC],
                        lhsT=w2f[:, ds_, ts(us_, P)],
                        rhs=qT[:, ds_, :],
                        start=(ds_ == 0),
                        stop=(ds_ == DS - 1),
                    )
                nc.vector.tensor_scalar_add(
                    bias_all[:, us_ * BPC : (us_ + 1) * BPC],
                    qp[:, us_ * BPC : (us_ + 1) * BPC],
                    b12c[:, us_ : us_ + 1],
                )
            if debug_dump:
                nc.sync.dma_start(out=bias_out[:, :], in_=bias_all)

            # ---- per batch ----
            for b in range(BPC):
                # load host-transposed bf16 values [d, t] (d on partitions)
                vT = vTp.tile([P, DS, T], bf16, tag="vT")
                vT_src = values_b16T[b].rearrange("(s p) t -> p s t", p=P)
                for tcb in range(TC):
                    nc.sync.dma_start(
                        out=vT[:, :, ts(tcb, TCH)],
                        in_=vT_src[:, :, ts(tcb, TCH)],
                    )
                # values fp32 (context operand) is needed only late
                vf32 = vf32p.tile([P, TS, D], f32r, tag="v")
                vsrc = values[b].rearrange("(s p) d -> p s d", p=P).bitcast(f32r)
                for ch in range(4):
                    eng = nc.gpsimd if ch % 2 == 0 else nc.scalar
                    eng.dma_start(
                        out=vf32[:, ch * 4 : (ch + 1) * 4, :],
                        in_=vsrc[:, ch * 4 : (ch + 1) * 4, :],
                    )

                if debug_dump and b == DBG_B:
                    nc.sync.dma_start(out=vT_out[:, :, :], in_=vT)
                echunks = []
                zparts = smallp.tile([1, TC], f32, tag="zp")
                for tcb in range(TC):
                    ths = []
                    for us_ in range(US):
                        pt = ppsum.tile([P, TCH], f32, tag="pt")
                        for ds_ in range(DS):
                            nc.tensor.matmul(
                                pt,
                                lhsT=w1b[:, ds_, ts(us_, P)],
                                rhs=vT[:, ds_, ts(tcb, TCH)],
                                start=(ds_ == 0),
                                stop=(ds_ == DS - 1),
                            )
                        th = thp.tile([P, TCH], f32r, tag="th")
                        nc.scalar.activation(
                            out=th,
                            in_=pt,
                            func=Act.Tanh,
                            bias=bias_all[:, us_ * BPC + b : us_ * BPC + b + 1],
                            scale=1.0,
                        )
                        ths.append(th)
                        if debug_dump and b == DBG_B:
                            nc.sync.dma_start(
                                out=th_out[tcb, us_].bitcast(f32r), in_=th
                            )
                    st = spsum.tile([1, TCH], f32, tag="st")
                    for us_ in range(US):
                        nc.tensor.matmul(
                            st,
                            lhsT=vcol[:, us_ : us_ + 1],
                            rhs=ths[us_],
                            start=(us_ == 0),
                            stop=(us_ == US - 1),
                        )
                    ech = ebp.tile([1, TCH], f32, tag="E")
                    nc.scalar.activation(
                        out=ech,
                        in_=st,
                        func=Act.Exp,
                        accum_out=zparts[:, tcb : tcb + 1],
                    )
                    echunks.append(ech)

                if debug_dump:
                    for tcb in range(TC):
                        nc.sync.dma_start(
                            out=e_out[b : b + 1, ts(tcb, TCH)], in_=echunks[tcb]
                        )
                z = smallp.tile([1, 1], f32, tag="z")
                nc.vector.reduce_sum(out=z, in_=zparts, axis=mybir.AxisListType.X)
                zi = smallp.tile([1, 1], f32, tag="zi")
                nc.vector.reciprocal(out=zi, in_=z)

                # attn = E / Z  -> DRAM
                at = smallp.tile([1, T], f32, tag="at")
                for tcb in range(TC):
                    nc.vector.tensor_scalar_mul(
                        at[:, ts(tcb, TCH)], echunks[tcb], zi
                    )
                (nc.sync if b == BPC - 1 else nc.scalar).dma_start(
                    out=attn_out[b].rearrange("t one -> one t"), in_=at
                )

                # E columns via K=1 rank-1 matmuls: ecol_ps[:, s] = E[s*128:+128]^T
                ecol_ps = epsum.tile([P, TS], f32, tag="ecps")
                for s in range(TS):
                    nc.tensor.matmul(
                        ecol_ps[:, s : s + 1],
                        lhsT=echunks[s // 4][:, ts(s % 4, P)],
                        rhs=ones8f[:, 0:1],
                        start=True,
                        stop=True,
                    )
                ecol = ecolp.tile([P, TS], f32r, tag="ecol")
                nc.scalar.activation(out=ecol, in_=ecol_ps, func=Act.Copy)

                # context_raw[d] = sum_t E[t] * values[t, d]
                ct = cpsum.tile([1, D], f32, tag="cps")
                for s in range(TS):
                    nc.tensor.matmul(
                        ct,
                        lhsT=ecol[:, s : s + 1],
                        rhs=vf32[:, s, :],
                        start=(s == 0),
                        stop=(s == TS - 1),
                    )
                co = smallp.tile([1, D], f32, tag="co")
                nc.scalar.activation(
                    out=co, in_=ct, func=Act.Copy, scale=zi
                )
                (nc.sync if b == BPC - 1 else nc.scalar).dma_start(
                    out=ctx_out[b : b + 1, :], in_=co
                )

    nc.compile()
    return nc


def _get_nc(debug_dump=False):
    key = ("nc", debug_dump)
    if key not in _cache:
        _cache[key] = _build_nc(debug_dump)
    return _cache[key]


def _run(inputs, trace=False, debug_dump=False, **kwargs):
    from concourse.bass_utils import run_bass_kernel_spmd

    nc = _get_nc(debug_dump)
    import ml_dtypes

    values = np.ascontiguousarray(np.asarray(inputs["values"], dtype=np.float32))
    values_b16T = np.ascontiguousarray(
        values.astype(ml_dtypes.bfloat16).transpose(0, 2, 1)
    )
    query = np.ascontiguousarray(np.asarray(inputs["query"], dtype=np.float32))
    w1 = np.ascontiguousarray(np.asarray(inputs["W1"], dtype=np.float32))
    b1 = np.ascontiguousarray(np.asarray(inputs["b1"], dtype=np.float32))
    w2 = np.ascontiguousarray(np.asarray(inputs["W2"], dtype=np.float32))
    b2 = np.ascontiguousarray(np.asarray(inputs["b2"], dtype=np.float32))
    v = np.ascontiguousarray(np.asarray(inputs["V"], dtype=np.float32))
    w1b16 = w1.astype(ml_dtypes.bfloat16)
    b12h = np.ascontiguousarray((b1 + b2).reshape(US, P).T)

    in_maps = []
    for c in range(NCORES):
        sl = slice(c * BPC, (c + 1) * BPC)
        qslice = query[sl]  # [BPC, D]
        # qTh[p, s*BPC+b] = query[b, s*128+p]
        qTh = np.ascontiguousarray(
            qslice.reshape(BPC, DS, P).transpose(2, 1, 0).reshape(P, DS * BPC)
        )
        vcolh = np.ascontiguousarray(v.reshape(US, P).T)
        in_maps.append(
            {
                "values": values[sl],
                "values_b16T": values_b16T[sl],
                "qTh": qTh,
                "vcolh": vcolh,
                "b12h": b12h,
                "W1b16": w1b16,
                "W2": w2,
            }
        )
    res = run_bass_kernel_spmd(
        nc, in_maps, core_ids=list(range(NCORES)), trace=trace, **kwargs
    )
    ctx = np.concatenate(
        [res.results[c]["context"] for c in range(NCORES)], axis=0
    )
    attn = np.concatenate(
        [res.results[c]["attn"] for c in range(NCORES)], axis=0
    )
    return (ctx, attn), res


def kernel(values, query, W1, b1, W2, b2, V, bv=None, **_unused):
    # bv cancels inside the softmax (softmax(s + c) == softmax(s)) and does
    # not appear anywhere else, so it is not shipped to the device.
    out, _ = _run(
        {
            "values": values,
            "query": query,
            "W1": W1,
            "b1": b1,
            "W2": W2,
            "b2": b2,
            "V": V,
        }
    )
    return out


# revision 41
# speedup vs baseline: 1.0017x; 1.0017x over previous
"""Bahdanau attention Trainium2 kernel.

reference:
    proj_v = values @ W1 + b1            # [B,T,U]
    proj_q = (query @ W2 + b2)[:,None,:] # [B,1,U]
    score  = tanh(proj_v + proj_q) @ V + bv
    attn   = softmax(score, axis=1)      # [B,T,1]
    context= sum(attn * values, axis=1)  # [B,D]

Sharding: data-parallel over batch, B=64 -> 8 batches per core on 8 cores.
Weights replicated.

Per-core dataflow (b = 8 batches, T=2048, D=U=512):
  - values arrive twice: fp32 natural [t, d] (context operand, read as f32r)
    and host-prepared bf16 transposed [d, t] (proj moving operand).  The
    transpose is done on the host because the on-device DMA-xbar transpose
    does not honor its semaphore waits reliably (verified race on HW).
  - proj: PE matmul, W1 (host-cast bf16) stationary, valuesT bf16 moving
    -> PSUM [u, t] tiles; 1 cycle/row.
  - proj_q (query @ W2, f32r) computed once; (b1+b2) added via DVE
    tensor_scalar into a per-(u, batch) bias table; the bias is applied by
    the ACT Tanh bias operand (fused add+tanh, PSUM -> SBUF f32r).
  - score: PE matmul, V column (f32r) stationary, tanh tile moving -> [1, t].
  - softmax without max subtraction (|score| <= ||V||_1 ~ 18, exp is safe in
    fp32; bv cancels in softmax and is ignored).  ACT Exp computes E and its
    accum_out gives the partial sums of Z for free.
  - E re-laid [1, T] -> [t, ts] columns with 16 K=1 rank-1 PE matmuls
    (cheaper and race-free vs a strided-DMA roundtrip); context: PE matmul,
    E column stationary, values fp32-as-f32r moving -> [1, D]; x(1/Z) on ACT.
  - DMA spread: vT on the SP HWDGE ring, values fp32 chunks alternating
    SWDGE (gpsimd) / Act HWDGE ring, setup tinies on SWDGE first.
"""

import sys

if "/opt/trn_rl_repo" not in sys.path:
    sys.path.insert(0, "/opt/trn_rl_repo")

import numpy as np

B, T, D, U = 64, 2048, 512, 512
NCORES = 8
BPC = B // NCORES  # batches per core

P = 128
DS = D // P   # 4  d slices
US = U // P   # 4  u slices
TS = T // P   # 16 t slices of 128
TCH = 512     # t chunk for matmul free dim
TC = T // TCH  # 4
DBG_B = 1

_cache = {}


def _build_nc(debug_dump=False):
    import concourse.bass as bass
    import concourse.tile as tile
    from concourse import bacc
    from concourse import mybir
    from concourse.bass import ts

    f32 = mybir.dt.float32
    f32r = mybir.dt.float32r
    bf16 = mybir.dt.bfloat16
    Act = mybir.ActivationFunctionType

    nc = bacc.Bacc("TRN2", target_bir_lowering=False, debug=False)

    values_b16n = nc.dram_tensor(
        "values_b16n", [BPC, T, D], bf16, kind="ExternalInput"
    )
    values_b16T = nc.dram_tensor(
        "values_b16T", [BPC, D, T], bf16, kind="ExternalInput"
    )
    qTh = nc.dram_tensor("qTh", [P, DS * BPC], f32, kind="ExternalInput")
    vcolh = nc.dram_tensor("vcolh", [P, US], f32, kind="ExternalInput")
    b12h = nc.dram_tensor("b12h", [P, US], f32, kind="ExternalInput")
    W1b16 = nc.dram_tensor("W1b16", [D, U], bf16, kind="ExternalInput")
    W2 = nc.dram_tensor("W2", [D, U], f32, kind="ExternalInput")
    ctx_out = nc.dram_tensor("context", [BPC, D], f32, kind="ExternalOutput")
    attn_out = nc.dram_tensor("attn", [BPC, T, 1], f32, kind="ExternalOutput")
    if debug_dump:
        vT_out = nc.dram_tensor("vT_dbg", [P, DS, T], bf16, kind="ExternalOutput")
        th_out = nc.dram_tensor("th_dbg", [TC, US, P, TCH], f32, kind="ExternalOutput")
        e_out = nc.dram_tensor("e_dbg", [BPC, T], f32, kind="ExternalOutput")
        bias_out = nc.dram_tensor("bias_dbg", [P, US * BPC], f32, kind="ExternalOutput")

    with tile.TileContext(nc) as tc:
        with (
            tc.tile_pool(name="consts", bufs=1) as consts,
            tc.tile_pool(name="vf32", bufs=2) as vf32p,
            tc.tile_pool(name="vT", bufs=2) as vTp,
            tc.tile_pool(name="th", bufs=8) as thp,
            tc.tile_pool(name="small", bufs=2) as smallp,
            tc.tile_pool(name="eb", bufs=10) as ebp,
            tc.tile_pool(name="ecol", bufs=2) as ecolp,
            tc.tile_pool(name="ppsum", bufs=3, space="PSUM") as ppsum,
            tc.tile_pool(name="spsum", bufs=2, space="PSUM") as spsum,
            tc.tile_pool(name="cpsum", bufs=2, space="PSUM") as cpsum,
            tc.tile_pool(name="epsum", bufs=1, space="PSUM") as epsum,
        ):
            # ---- weights / setup ----
            w1b = consts.tile([P, DS, U], bf16)
            w1src = W1b16.rearrange("(s p) u -> p s u", p=P)
            for us_ in range(US):
                nc.scalar.dma_start(
                    out=w1b[:, :, ts(us_, P)], in_=w1src[:, :, ts(us_, P)]
                )

            vcol = consts.tile([P, US], f32r)
            qT = consts.tile([P, DS, BPC], f32r)
            b12c = consts.tile([P, US], f32)
            nc.gpsimd.dma_start(out=vcol, in_=vcolh[:, :].bitcast(f32r))
            nc.gpsimd.dma_start(
                out=qT, in_=qTh.rearrange("p (s b) -> p s b", b=BPC).bitcast(f32r)
            )
            nc.gpsimd.dma_start(out=b12c, in_=b12h[:, :])
            w2f = consts.tile([P, DS, U], f32r)
            nc.gpsimd.dma_start(
                out=w2f, in_=W2.rearrange("(s p) u -> p s u", p=P).bitcast(f32r)
            )
            ones8f = consts.tile([1, BPC], f32)
            nc.vector.memset(ones8f, 1.0)

            # bias_all[u, (us,b)] = proj_q[u, b] + (b1+b2)[u]
            bias_all = consts.tile([P, US * BPC], f32)
            qp = cpsum.tile([P, US * BPC], f32, tag="cps")
            for us_ in range(US):
                for ds_ in range(DS):
                    nc.tensor.matmul(
                        qp[:, us_ * BPC : (us_ + 1) * BPC],
                        lhsT=w2f[:, ds_, ts(us_, P)],
                        rhs=qT[:, ds_, :],
                        start=(ds_ == 0),
                        stop=(ds_ == DS - 1),
                    )
                nc.vector.tensor_scalar_add(
                    bias_all[:, us_ * BPC : (us_ + 1) * BPC],
                    qp[:, us_ * BPC : (us_ + 1) * BPC],
                    b12c[:, us_ : us_ + 1],
                )
            if debug_dump:
                nc.sync.dma_start(out=bias_out[:, :], in_=bias_all)

            # ---- per batch ----
            for b in range(BPC):
                # load host-transposed bf16 values [d, t] (d on partitions)
                vT = vTp.tile([P, DS, T], bf16, tag="vT")
                vT_src = values_b16T[b].rearrange("(s p) t -> p s t", p=P)
                for tcb in range(TC):
                    nc.sync.dma_start(
                        out=vT[:, :, ts(tcb, TCH)],
                        in_=vT_src[:, :, ts(tcb, TCH)],
                    )
                # values bf16 natural (context operand) is needed only late
                vbn = vf32p.tile([P, TS, D], bf16, tag="v")
                vsrc = values_b16n[b].rearrange("(s p) d -> p s d", p=P)
                for ch in range(4):
                    eng = nc.gpsimd if ch % 2 == 0 else nc.scalar
                    eng.dma_start(
                        out=vbn[:, ch * 4 : (ch + 1) * 4, :],
                        in_=vsrc[:, ch * 4 : (ch + 1) * 4, :],
                    )

                if debug_dump and b == DBG_B:
                    nc.sync.dma_start(out=vT_out[:, :, :], in_=vT)
                echunks = []
                zparts = smallp.tile([1, TC], f32, tag="zp")
                for tcb in range(TC):
                    ths = []
                    for us_ in range(US):
                        pt = ppsum.tile([P, TCH], f32, tag="pt")
                        for ds_ in range(DS):
                            nc.tensor.matmul(
                                pt,
                                lhsT=w1b[:, ds_, ts(us_, P)],
                                rhs=vT[:, ds_, ts(tcb, TCH)],
                                start=(ds_ == 0),
                                stop=(ds_ == DS - 1),
                            )
                        th = thp.tile([P, TCH], f32r, tag="th")
                        nc.scalar.activation(
                            out=th,
                            in_=pt,
                            func=Act.Tanh,
                            bias=bias_all[:, us_ * BPC + b : us_ * BPC + b + 1],
                            scale=1.0,
                        )
                        ths.append(th)
                        if debug_dump and b == DBG_B:
                            nc.sync.dma_start(
                                out=th_out[tcb, us_].bitcast(f32r), in_=th
                            )
                    st = spsum.tile([1, TCH], f32, tag="st")
                    for us_ in range(US):
                        nc.tensor.matmul(
                            st,
                            lhsT=vcol[:, us_ : us_ + 1],
                            rhs=ths[us_],
                            start=(us_ == 0),
                            stop=(us_ == US - 1),
                        )
                    ech = ebp.tile([1, TCH], f32, tag="E")
                    nc.scalar.activation(
                        out=ech,
                        in_=st,
                        func=Act.Exp,
                        accum_out=zparts[:, tcb : tcb + 1],
                    )
                    echunks.append(ech)

                if debug_dump:
                    for tcb in range(TC):
                        nc.sync.dma_start(
                            out=e_out[b : b + 1, ts(tcb, TCH)], in_=echunks[tcb]
                        )
                z = smallp.tile([1, 1], f32, tag="z")
                nc.vector.reduce_sum(out=z, in_=zparts, axis=mybir.AxisListType.X)
                zi = smallp.tile([1, 1], f32, tag="zi")
                nc.vector.reciprocal(out=zi, in_=z)

                # attn = E / Z  -> DRAM
                at = smallp.tile([1, T], f32, tag="at")
                for tcb in range(TC):
                    nc.vector.tensor_scalar_mul(
                        at[:, ts(tcb, TCH)], echunks[tcb], zi
                    )
                (nc.sync if b == BPC - 1 else nc.scalar).dma_start(
                    out=attn_out[b].rearrange("t one -> one t"), in_=at
                )

                # E columns via K=1 rank-1 matmuls: ecol_ps[:, s] = E[s*128:+128]^T
                ecol_ps = epsum.tile([P, TS], f32, tag="ecps")
                for s in range(TS):
                    nc.tensor.matmul(
                        ecol_ps[:, s : s + 1],
                        lhsT=echunks[s // 4][:, ts(s % 4, P)],
                        rhs=ones8f[:, 0:1],
                        start=True,
                        stop=True,
                    )
                ecol = ecolp.tile([P, TS], bf16, tag="ecol")
                nc.scalar.activation(out=ecol, in_=ecol_ps, func=Act.Copy)

                # context_raw[d] = sum_t E[t] * values[t, d]
                ct = cpsum.tile([1, D], f32, tag="cps")
                for s in range(TS):
                    nc.tensor.matmul(
                        ct,
                        lhsT=ecol[:, s : s + 1],
                        rhs=vbn[:, s, :],
                        start=(s == 0),
                        stop=(s == TS - 1),
                    )
                co = smallp.tile([1, D], f32, tag="co")
                nc.scalar.activation(
                    out=co, in_=ct, func=Act.Copy, scale=zi
                )
                (nc.sync if b == BPC - 1 else nc.scalar).dma_start(
                    out=ctx_out[b : b + 1, :], in_=co
                )

    nc.compile()
    return nc


def _get_nc(debug_dump=False):
    key = ("nc", debug_dump)
    if key not in _cache:
        _cache[key] = _build_nc(debug_dump)
    return _cache[key]


def _run(inputs, trace=False, debug_dump=False, **kwargs):
    from concourse.bass_utils import run_bass_kernel_spmd

    nc = _get_nc(debug_dump)
    import ml_dtypes

    values = np.ascontiguousarray(np.asarray(inputs["values"], dtype=np.float32))
    values_b16n = values.astype(ml_dtypes.bfloat16)
    values_b16T = np.ascontiguousarray(values_b16n.transpose(0, 2, 1))
    query = np.ascontiguousarray(np.asarray(inputs["query"], dtype=np.float32))
    w1 = np.ascontiguousarray(np.asarray(inputs["W1"], dtype=np.float32))
    b1 = np.ascontiguousarray(np.asarray(inputs["b1"], dtype=np.float32))
    w2 = np.ascontiguousarray(np.asarray(inputs["W2"], dtype=np.float32))
    b2 = np.ascontiguousarray(np.asarray(inputs["b2"], dtype=np.float32))
    v = np.ascontiguousarray(np.asarray(inputs["V"], dtype=np.float32))
    w1b16 = w1.astype(ml_dtypes.bfloat16)
    b12h = np.ascontiguousarray((b1 + b2).reshape(US, P).T)

    in_maps = []
    for c in range(NCORES):
        sl = slice(c * BPC, (c + 1) * BPC)
        qslice = query[sl]  # [BPC, D]
        # qTh[p, s*BPC+b] = query[b, s*128+p]
        qTh = np.ascontiguousarray(
            qslice.reshape(BPC, DS, P).transpose(2, 1, 0).reshape(P, DS * BPC)
        )
        vcolh = np.ascontiguousarray(v.reshape(US, P).T)
        in_maps.append(
            {
                "values_b16n": values_b16n[sl],
                "values_b16T": values_b16T[sl],
                "qTh": qTh,
                "vcolh": vcolh,
                "b12h": b12h,
                "W1b16": w1b16,
                "W2": w2,
            }
        )
    res = run_bass_kernel_spmd(
        nc, in_maps, core_ids=list(range(NCORES)), trace=trace, **kwargs
    )
    ctx = np.concatenate(
        [res.results[c]["context"] for c in range(NCORES)], axis=0
    )
    attn = np.concatenate(
        [res.results[c]["attn"] for c in range(NCORES)], axis=0
    )
    return (ctx, attn), res


def kernel(values, query, W1, b1, W2, b2, V, bv=None, **_unused):
    # bv cancels inside the softmax (softmax(s + c) == softmax(s)) and does
    # not appear anywhere else, so it is not shipped to the device.
    out, _ = _run(
        {
            "values": values,
            "query": query,
            "W1": W1,
            "b1": b1,
            "W2": W2,
            "b2": b2,
            "V": V,
        }
    )
    return out
